# revision 1
# baseline (speedup 1.0000x reference)
"""Trainium2 Bass kernel v2 for nn_Attention_40759239639448.

Contract: kernel(**inputs) takes full inputs, returns full [B, T, C] output.
8-core tensor-parallel over heads (2 heads/core, both batches); host sums
the 8 bf16 partial out-projections.

v2 design:
 - single merged loop, software-pipelined one chunk ahead: QKV+RoPE+RMS for
   chunk ci+1 issues before attention q-tile ci, so PE never waits on the
   rope DVE chain.
 - bf16 off the fp32 PSUM accumulations: x, weights, qT/kT, P, vaug, yT,
   out partials.
 - rope: DVE stream_shuffle (32-partition-block swap) + sign-folded sin
   tables; combine add + sin-mul offloaded to the idle GpSimd/Pool engine.
 - RMS rsqrt: block-identity matmul broadcasts sum(q^2) to all 64 head
   rows, then ACT Ln + Exp(-0.5*) - one ACT table set for the whole kernel
   (activation tables patched so exp/ln/copy resolve only to
   natural_log_exp_and_others - no table thrash).
 - causal trimming: S / exp / PV restricted to valid columns on diagonal
   tiles; one 128x128 triangle table masks only the diagonal block.
 - denominator via ones-column in vaug (65-wide PV lhsT); per-q-tile
   normalization (copy -> reciprocal -> tiny broadcast matmul -> muls) and
   per-q-tile out-projection + single batched output DMA.
 - DMA batching: one input DMA per chunk, one output DMA per q-tile.
"""
import sys

sys.path.insert(0, "/opt/trn_rl_repo")

import numpy as np
import ml_dtypes

BF16 = ml_dtypes.bfloat16

B, T, C, H = 2, 2048, 1024, 16
D = C // H
NC = 8
TT = B * T
EPS = 1e-6
ROPE_BASE = 10000.0
NCH = 8
CH = 512
KT = 128
QT = 512
NKT = T // KT
SC = 0.125

_cache = {}


def _patch_act_tables():
    """Route {exp, ln, square, copy, identity, memset_zero} exclusively to
    natural_log_exp_and_others so the table-load pass can't thrash between
    sets. Names/indices preserved; only membership shrinks."""
    import concourse.hw_specs as hw_specs
    import concourse.bacc as bacc
    import concourse.mybir as mybir

    AF = mybir.ActivationFunctionType
    mine = {AF.Exp, AF.Ln, AF.Square, AF.Copy, AF.Identity, AF.MemsetZero}
    orig = hw_specs.get_activation_tables
    if getattr(hw_specs, "_act_tables_patched", False):
        return

    def patched(module_arch):
        tables = orig(module_arch)
        if not any(
            name == "natural_log_exp_and_others" and mine <= fns
            for name, fns in tables.items()
        ):
            return tables
        return {
            name: (fns if name == "natural_log_exp_and_others" else fns - mine)
            for name, fns in tables.items()
        }

    hw_specs.get_activation_tables = patched
    bacc.get_activation_tables = patched
    hw_specs._act_tables_patched = True


def _build():
    _patch_act_tables()
    import concourse.bacc as bacc
    import concourse.mybir as mybir
    import concourse.tile as tile

    f32 = mybir.dt.float32
    f32r = mybir.dt.float32r
    bf16 = mybir.dt.bfloat16
    AF = mybir.ActivationFunctionType

    nc = bacc.Bacc(None, target_bir_lowering=False)

    xT_d = nc.dram_tensor("xT", [C, TT], bf16, kind="ExternalInput")
    wqT_d = nc.dram_tensor("wqT", [C, 128], bf16, kind="ExternalInput")
    wkT_d = nc.dram_tensor("wkT", [C, 128], bf16, kind="ExternalInput")
    wvT_d = nc.dram_tensor("wvT", [C, 128], bf16, kind="ExternalInput")
    woT_d = nc.dram_tensor("woT", [128, C], bf16, kind="ExternalInput")
    cosq_d = nc.dram_tensor("cosq", [128, T], f32, kind="ExternalInput")
    sinq_d = nc.dram_tensor("sinq", [128, T], f32, kind="ExternalInput")
    cosk_d = nc.dram_tensor("cosk", [128, T], f32, kind="ExternalInput")
    sink_d = nc.dram_tensor("sink", [128, T], f32, kind="ExternalInput")
    mtri_d = nc.dram_tensor("mtri", [128, 128], bf16, kind="ExternalInput")
    ee_d = nc.dram_tensor("ee", [128, 128], f32r, kind="ExternalInput")
    ident_d = nc.dram_tensor("ident", [128, 128], bf16, kind="ExternalInput")
    onesc_d = nc.dram_tensor("onesc", [1, 64], f32r, kind="ExternalInput")
    epsb_d = nc.dram_tensor("epsb", [128, 1], f32, kind="ExternalInput")
    out_d = nc.dram_tensor("out", [TT, C], bf16, kind="ExternalOutput")

    # rope-pair partner is 16 partitions away inside each 32-quadrant
    # (host permutes the qk d-channel order to arrange this)
    SHUF_MASK = [i ^ 16 for i in range(32)]

    with tile.TileContext(nc) as tc:
        with (
            tc.tile_pool(name="persist", bufs=1) as pp,
            tc.tile_pool(name="xp", bufs=3) as xp,
            tc.tile_pool(name="scr", bufs=8) as scr,
            tc.tile_pool(name="scrv", bufs=2) as scrv,
            tc.tile_pool(name="scr2", bufs=2) as scr2,
            tc.tile_pool(name="pbuf", bufs=10) as pbuf,
            tc.tile_pool(name="ysc", bufs=3) as ysc,
            tc.tile_pool(name="osb", bufs=3) as osb,
            tc.tile_pool(name="ps_c", bufs=2, space="PSUM") as ps_c,
            tc.tile_pool(name="ps_s", bufs=2, space="PSUM") as ps_s,
            tc.tile_pool(name="ps_y", bufs=1, space="PSUM") as ps_y,
        ):
            qT = pp.tile([128, TT], bf16, tag="qT")
            kT = pp.tile([128, TT], bf16, tag="kT")
            vaug = pp.tile([128, B * NKT * 130], bf16, tag="vaug")
            wq_sb = pp.tile([128, C], bf16, tag="wq")
            wk_sb = pp.tile([128, C], bf16, tag="wk")
            wv_sb = pp.tile([128, C], bf16, tag="wv")
            wo_sb = pp.tile([128, C], bf16, tag="wo")
            cosq = pp.tile([128, T], f32, tag="cosq")
            sinq = pp.tile([128, T], f32, tag="sinq")
            cosk = pp.tile([128, T], f32, tag="cosk")
            sink = pp.tile([128, T], f32, tag="sink")
            mtri = pp.tile([128, 128], bf16, tag="mtri")
            ee = pp.tile([128, 128], f32r, tag="ee")
            ident = pp.tile([128, 128], bf16, tag="ident")
            onesc = pp.tile([1, 64], f32r, tag="onesc")
            epsb = pp.tile([128, 1], f32, tag="epsb")

            def wload(dst, src):
                nc.sync.dma_start(
                    dst[:].rearrange("p (g c) -> p g c", g=8),
                    src[:].rearrange("(g p) c -> p g c", p=128),
                )

            # SP queue carries x + qkv weights (critical path); constants ride
            # the idle ACT queue; trig tables load staggered per-chunk on the
            # Pool queue inside phase1. Kills the DMA-only startup.
            nc.scalar.dma_start(
                wq_sb[:].rearrange("p (g c) -> p g c", g=8),
                wqT_d[:].rearrange("(g p) c -> p g c", p=128),
            )
            nc.scalar.dma_start(
                wk_sb[:].rearrange("p (g c) -> p g c", g=8),
                wkT_d[:].rearrange("(g p) c -> p g c", p=128),
            )
            nc.scalar.dma_start(
                wv_sb[:].rearrange("p (g c) -> p g c", g=8),
                wvT_d[:].rearrange("(g p) c -> p g c", p=128),
            )
            nc.scalar.dma_start(epsb[:], epsb_d[:])
            nc.scalar.dma_start(ee[:], ee_d[:])
            nc.scalar.dma_start(ident[:], ident_d[:])
            nc.scalar.dma_start(mtri[:], mtri_d[:])
            nc.scalar.dma_start(onesc[:], onesc_d[:])
            nc.scalar.dma_start(wo_sb[:], woT_d[:])
            nc.gpsimd.memset(
                vaug[:].rearrange("p (k c) -> p k c", c=65)[:, :, 64], 1.0
            )

            def phase1_groups(ci):
                """Issue-groups (closures) for chunk ci's QKV+RoPE+RMS, to be
                interleaved with the concurrent attention k-loop. Engine
                program order per group keeps PE fed: matmuls first, the
                rope/rms consumers trail on DVE/Pool/ACT."""
                t0 = ci * CH
                tt0 = t0 % T
                bb = t0 // T
                xt = xp.tile([128, 8, CH], bf16, tag="x")
                xsrc = xT_d[:, t0 : t0 + CH].rearrange("(g p) t -> p g t", p=128)
                nc.sync.dma_start(xt[:, 0:4, :], xsrc[:, 0:4, :])
                nc.sync.dma_start(xt[:, 4:8, :], xsrc[:, 4:8, :])
                if ci < 4:  # both batches share the same trig columns
                    tsl = slice(tt0, tt0 + CH)
                    nc.gpsimd.dma_start(cosq[:, tsl], cosq_d[:, tsl])
                    nc.gpsimd.dma_start(sinq[:, tsl], sinq_d[:, tsl])
                    nc.gpsimd.dma_start(cosk[:, tsl], cosk_d[:, tsl])
                    nc.gpsimd.dma_start(sink[:, tsl], sink_d[:, tsl])

                state = {}

                def proj_a(w_sb, name):
                    def go():
                        ps = ps_c.tile([128, CH], f32, tag="c")
                        state[name] = ps
                        for cc in range(4):
                            nc.tensor.matmul(
                                ps[:], w_sb[:, 128 * cc : 128 * cc + 128],
                                xt[:, cc, :], start=(cc == 0), stop=False,
                            )
                    return go

                def proj(w_sb, name):
                    def go():
                        ps = state[name]
                        for cc in range(4, 8):
                            nc.tensor.matmul(
                                ps[:], w_sb[:, 128 * cc : 128 * cc + 128],
                                xt[:, cc, :], start=False, stop=(cc == 7),
                            )
                        if name != "v":
                            cos_t, sin_t = (
                                (cosq, sinq) if name == "q" else (cosk, sink)
                            )
                            xs = scr.tile([128, CH], f32, tag="s")
                            nc.vector.stream_shuffle(xs[:], ps[:], SHUF_MASK)
                            tc_ = scr.tile([128, CH], f32, tag="s")
                            nc.vector.tensor_mul(
                                tc_[:], ps[:], cos_t[:, tt0 : tt0 + CH]
                            )
                            # sum of squares is pair-permutation invariant,
                            # so square the shuffled SBUF copy (frees psum)
                            sq = scr.tile([128, CH], f32r, tag="s")
                            with nc.allow_low_precision(reason="rms f32r"):
                                nc.vector.tensor_mul(sq[:], xs[:], xs[:])
                            ts2 = scr.tile([128, CH], f32, tag="s")
                            nc.gpsimd.tensor_mul(
                                ts2[:], xs[:], sin_t[:, tt0 : tt0 + CH]
                            )
                            o_ = scr.tile([128, CH], f32, tag="s")
                            nc.gpsimd.tensor_add(o_[:], tc_[:], ts2[:])
                            state[name + "_sq"] = sq
                            state[name + "_o"] = o_
                        else:
                            v_sb = scrv.tile([128, CH], bf16, tag="v")
                            # pops during attn(ci-1): short k-loops leave ACT
                            # idle, long ones are exp-paced - choose engine
                            if (ci + 3) % 4 < 2:
                                nc.scalar.copy(v_sb[:], ps[:])
                            else:
                                nc.vector.tensor_copy(v_sb[:], ps[:])
                            state["v_sb"] = v_sb

                    return go

                def rms_and_vt():
                    # block-identity matmuls broadcast sum(q^2) per head;
                    # q and k share one tile so ln/exp run once at full width
                    ms_ps = ps_s.tile([128, 2 * CH], f32, tag="s")
                    for i, name in enumerate(("q", "k")):
                        nc.tensor.matmul(
                            ms_ps[:, i * CH : i * CH + CH],
                            ee[:], state[name + "_sq"][:],
                            start=True, stop=True,
                        )
                    state["ms"] = ms_ps
                    for ti in range(4):
                        vt_ps = ps_c.tile([128, 128], bf16, tag="c")
                        nc.tensor.transpose(
                            vt_ps[:],
                            state["v_sb"][:, ti * 128 : ti * 128 + 128],
                            ident[:],
                        )
                        kb = bb * NKT + (tt0 // KT) + ti
                        dst = vaug[:, kb * 130 : kb * 130 + 130].rearrange(
                            "p (k c) -> p k c", c=65
                        )[:, :, 0:64]
                        nc.vector.tensor_copy(
                            dst, vt_ps[:].rearrange("p (k c) -> p k c", c=64)
                        )

                def norm_store():
                    ln_ = scr2.tile([128, 2 * CH], f32, tag="s2")
                    nc.scalar.activation(
                        ln_[:], state["ms"][:], AF.Ln,
                        scale=1.0 / D, bias=epsb[:],
                    )
                    rs_ = scr2.tile([128, 2 * CH], f32, tag="s2")
                    nc.scalar.activation(rs_[:], ln_[:], AF.Exp, scale=-0.5)
                    for i, (name, dst) in enumerate((("q", qT), ("k", kT))):
                        nc.vector.tensor_mul(
                            dst[:, t0 : t0 + CH], state[name + "_o"][:],
                            rs_[:, i * CH : i * CH + CH],
                        )

                return [
                    proj_a(wq_sb, "q"),
                    proj(wq_sb, "q"),
                    proj_a(wk_sb, "k"),
                    proj(wk_sb, "k"),
                    proj_a(wv_sb, "v"),
                    proj(wv_sb, "v"),
                    rms_and_vt,
                    norm_store,
                ]

            def attn(bb, qi, fillers):
                """Attention k-loop for (bb, qi), popping one filler closure
                (next-chunk matmul group / prev-tile tail step) after each
                (s_exp, pv) pair so PE always has independent work queued
                behind the exp-gated PV matmuls."""
                bt = bb * T
                q0 = bt + qi * QT
                nk = 4 * qi + 4
                y_ps = ps_y.tile([65, 2 * QT], f32, tag="y")
                p_tiles = [None] * nk
                fillers = list(fillers)

                def fill():
                    if fillers:
                        fillers.pop(0)()

                def s_exp(ki):
                    mi = ki - 4 * qi
                    off = max(mi, 0) * KT
                    diag = mi >= 0
                    k0 = bt + ki * KT
                    s_ps = ps_s.tile([128, 2 * QT], f32, tag="s")
                    for h in range(2):
                        hs = slice(64 * h, 64 * h + 64)
                        nc.tensor.matmul(
                            s_ps[:, h * QT + off : h * QT + QT],
                            kT[hs, k0 : k0 + KT],
                            qT[hs, q0 + off : q0 + QT],
                            start=True, stop=not diag,
                            tile_position=(64 * h, 0),
                        )
                    if diag:
                        # causal mask: accumulate -1e9 strict-lower triangle
                        # into the diagonal 128x128 block, exp then yields 0
                        for h in range(2):
                            nc.tensor.matmul(
                                s_ps[:, h * QT + off : h * QT + off + KT],
                                mtri[:], ident[:],
                                start=False, stop=True,
                            )
                    p_sb = pbuf.tile([128, 2 * QT], bf16, tag="p")
                    if off == 0:
                        nc.scalar.activation(p_sb[:], s_ps[:], AF.Exp, scale=SC)
                    else:
                        nc.scalar.activation(
                            p_sb[:].rearrange("p (h q) -> p h q", h=2)[
                                :, :, off:QT
                            ],
                            s_ps[:].rearrange("p (h q) -> p h q", h=2)[
                                :, :, off:QT
                            ],
                            AF.Exp, scale=SC,
                        )
                    p_tiles[ki] = p_sb

                def pv(ki):
                    mi = ki - 4 * qi
                    off = max(mi, 0) * KT
                    kb = bb * NKT + ki
                    for h in range(2):
                        nc.tensor.matmul(
                            y_ps[:, h * QT + off : h * QT + QT],
                            vaug[:, kb * 130 + 65 * h : kb * 130 + 65 * h + 65],
                            p_tiles[ki][:, h * QT + off : h * QT + QT],
                            start=(ki == 0), stop=(ki == nk - 1),
                        )

                # spread fillers evenly over the k-loop's fill slots
                nslots = nk + 1
                nf = len(fillers)
                sched = {}
                for i in range(nf):
                    s = min(nslots - 1, (i * nslots) // max(nf, 1))
                    sched[s] = sched.get(s, 0) + 1
                slot = 0

                def fill_at():
                    nonlocal slot
                    for _ in range(sched.get(slot, 0)):
                        fill()
                    slot += 1

                s_exp(0)
                if nk > 1:
                    s_exp(1)
                fill_at()
                for ki in range(2, nk):
                    s_exp(ki)
                    fill_at()
                    pv(ki - 2)
                fill_at()
                pv(nk - 2) if nk > 1 else None
                pv(nk - 1)
                while fillers:
                    fill()

                # free y_ps immediately; db/norm/outproj defer as fillers
                yraw = ysc.tile([65, 2 * QT], f32r, tag="yraw")
                # den row via ACT (idle at k-loop end) so the reciprocal
                # isn't queued behind DVE work; y rows follow on DVE, or ACT
                # when the next iteration's k-loop is short (ACT idle there)
                nc.scalar.copy(yraw[64:65, :], y_ps[64:65, :])
                if qi == 3:
                    nc.scalar.copy(yraw[0:64, :], y_ps[0:64, :])
                else:
                    nc.vector.tensor_copy(yraw[0:64, :], y_ps[0:64, :])
                rcp = ysc.tile([1, 2 * QT], f32r, tag="yTq")
                with nc.allow_low_precision(reason="denominator recip"):
                    nc.vector.reciprocal(rcp[:], yraw[64:65, :])

                def tail_norm():
                    db_ps = ps_y.tile([64, 2 * QT], f32, tag="y")
                    for h in range(2):
                        nc.tensor.matmul(
                            db_ps[:, h * QT : h * QT + QT],
                            onesc[:],
                            rcp[:, h * QT : h * QT + QT],
                            start=True, stop=True,
                        )
                    yTq = ysc.tile([128, QT], bf16, tag="yTq")
                    for h in range(2):
                        nc.vector.tensor_mul(
                            yTq[64 * h : 64 * h + 64, :],
                            yraw[0:64, h * QT : h * QT + QT],
                            db_ps[:, h * QT : h * QT + QT],
                        )
                    state_t["yTq"] = yTq

                def tail_proj():
                    yTq = state_t["yTq"]
                    last = bb == 1 and qi == 3
                    o_sb = osb.tile([128, 4 * C], bf16, tag="o")
                    for tt in range(4):
                        for half in range(2):
                            o_ps = ps_c.tile([128, 512], f32, tag="c")
                            nc.tensor.matmul(
                                o_ps[:],
                                yTq[:, tt * 128 : tt * 128 + 128],
                                wo_sb[:, half * 512 : half * 512 + 512],
                                start=True, stop=True,
                            )
                            dsl = o_sb[
                                :, tt * C + half * 512 : tt * C + half * 512 + 512
                            ]
                            # copies pop two iterations later: short k-loops
                            # (popping qi in {0,1} <=> own qi in {2,3}) leave
                            # ACT idle, so shift more copies onto it there
                            on_act = (
                                half == 0 if (last or qi >= 2)
                                else (tt in (1, 3) and half == 0)
                            )
                            if on_act:
                                nc.scalar.copy(dsl, o_ps[:])
                            else:
                                nc.vector.tensor_copy(dsl, o_ps[:])
                        nc.gpsimd.dma_start(
                            out_d[q0 + tt * 128 : q0 + tt * 128 + 128, :],
                            o_sb[:, tt * C : tt * C + C],
                        )

                state_t = {}
                return [tail_norm, tail_proj]

            for g in phase1_groups(0):
                g()
            tail = []
            proj_q = []  # out-projections deferred one extra iteration
            for ci in range(NCH):
                p1 = phase1_groups(ci + 1) if ci + 1 < NCH else []
                # interleave: PE-heavy projection groups first; previous
                # tile's db/norm and the (one-older) out-projection threaded
                # between them so even the last k-loop has PE filler
                fillers = []
                if p1:
                    fillers.extend(p1[0:4])   # q, k projections (split)
                fillers.extend(tail[0:1])     # db + normalize muls
                if p1:
                    fillers.extend(p1[4:7])   # v projection, rms/vt matmuls
                fillers.extend(proj_q)        # out-projection from ci-2
                proj_q = list(tail[1:2])
                if p1:
                    # rms ACT ops last so they don't delay exp-freed psum
                    fillers.append(p1[7])
                tail = attn(ci // 4, ci % 4, fillers)
            for g in tail[0:1] + proj_q + tail[1:2]:
                g()

    nc.finalize()
    return nc


def _host_prep(x, w_qkv, w_out, q_norm_w, k_norm_w):
    xT = np.ascontiguousarray(x.reshape(TT, C).T.astype(BF16))

    j = np.arange(32, dtype=np.float64)
    inv = ROPE_BASE ** (-j / 32.0)
    tt = np.arange(T, dtype=np.float64)
    ang = tt[:, None] * inv[None, :]
    cos_t = np.cos(ang)
    sin_t = np.sin(ang)

    # d-channel order per head: rope pair (d, d+32) sits 16 partitions
    # apart within one 32-partition quadrant (stream_shuffle i^16 reaches it)
    dmap = np.r_[0:16, 32:48, 16:32, 48:64]

    def trig_tables(w):
        w = np.asarray(w, dtype=np.float64)
        cosr = np.empty((128, T), np.float32)
        sinr = np.empty((128, T), np.float32)
        for p in range(128):
            d = dmap[p % 64]
            jj = d % 32
            sign = -1.0 if d < 32 else 1.0
            cosr[p] = (cos_t[:, jj] * w[d]).astype(np.float32)
            sinr[p] = (sign * sin_t[:, jj] * w[d]).astype(np.float32)
        return cosr, sinr

    cosq, sinq = trig_tables(q_norm_w)
    cosk, sink = trig_tables(k_norm_w)

    # mtri[i, j] = -1e9 for i < j: lhsT of the causal-bias matmul
    # (M = mtri^T @ I has M[k, c] = -1e9 where c < k)
    mtri = np.where(
        np.arange(128)[:, None] < np.arange(128)[None, :], -1e9, 0.0
    ).astype(BF16)
    ee = np.zeros((128, 128), np.float32)
    ee[0:64, 0:64] = 1.0
    ee[64:128, 64:128] = 1.0
    ident = np.eye(128, dtype=np.float32).astype(BF16)

    shared = {
        "xT": xT, "cosq": cosq, "sinq": sinq, "cosk": cosk, "sink": sink,
        "mtri": mtri, "ee": ee, "ident": ident,
        "onesc": np.ones((1, 64), np.float32),
        "epsb": np.full((128, 1), EPS, np.float32),
    }

    in_maps = []
    for c in range(NC):
        rows = np.arange(2 * c * 64, 2 * c * 64 + 128)
        qk_rows = np.concatenate([rows[0:64][dmap], rows[64:128][dmap]])
        m = dict(shared)
        m["wqT"] = np.ascontiguousarray(w_qkv[qk_rows, :].T.astype(BF16))
        m["wkT"] = np.ascontiguousarray(w_qkv[C + qk_rows, :].T.astype(BF16))
        m["wvT"] = np.ascontiguousarray(w_qkv[2 * C + rows, :].T.astype(BF16))
        m["woT"] = np.ascontiguousarray(w_out[:, rows].T.astype(BF16))
        in_maps.append(m)
    return in_maps


def kernel(x, w_qkv, w_out, q_norm_w, k_norm_w, _trace=False):
    from concourse.bass_utils import run_bass_kernel_spmd

    if "nc" not in _cache:
        _cache["nc"] = _build()
    nc = _cache["nc"]

    x = np.asarray(x, dtype=np.float32)
    w_qkv = np.asarray(w_qkv, dtype=np.float32)
    w_out = np.asarray(w_out, dtype=np.float32)
    q_norm_w = np.asarray(q_norm_w, dtype=np.float32)
    k_norm_w = np.asarray(k_norm_w, dtype=np.float32)

    in_maps = _host_prep(x, w_qkv, w_out, q_norm_w, k_norm_w)
    res = run_bass_kernel_spmd(nc, in_maps, list(range(NC)), trace=_trace)
    _cache["last_result"] = res
    out = np.zeros((TT, C), np.float64)
    for r in res.results:
        out += np.asarray(r["out"]).astype(np.float64)
    return out.astype(np.float32).reshape(B, T, C)

